# revision 59
# baseline (speedup 1.0000x reference)
"""GCNConv (message passing + linear) on 8 Trainium2 NeuronCores.

Strategy (graph/data parallel, per sharding hint):
  - Nodes sorted by (table, A-count, snake) and dealt round-robin to the
    8 cores (core c owns sorted-ranks {s : s % 8 == c}).
  - Node features staged as two bf16 DRAM tables (int16 gather index
    limit); each core bulk row-gathers per-edge source rows with the Q7
    dma_gather instruction (256B descriptors, 1024 idx/call, 4 SWDGE
    queues), then fuses the rsqrt(deg_src) scale with the cast to the
    message tiles on the DVE (per-edge degree counts uploaded as
    integer-valued tables; all rsqrt/normalization math on device).
  - Segment-sum on the TensorEngine: message tile [128 slots, 128 feat]
    (stationary) x diagonal rsqrt(deg_dst) matrix (streaming)
    accumulated into PSUM - one slot per owned dst per tile; self-loop
    contributions enter via a sequentially-loaded x_own tile (keeps
    them off the gather path).
  - Group batches processed largest-count-first so the long PE chains
    drain while gathers still stream.
  - Final linear via W^T matmul + bias; output is [d_out, local_dst];
    host unpermutes/transposes back to [N, d_out].

The Bass program is rebuilt per distinct edge_index (layout constants are
baked into the instruction stream); all 8 cores share one program and
differ only in their input data.
"""

import ml_dtypes
import numpy as np

_BF16 = ml_dtypes.bfloat16

import concourse.bacc as bacc
import concourse.mybir as mybir
import concourse.tile as tile
from concourse.bass_utils import run_bass_kernel_spmd
from concourse.library_config import mlp as _mlp_lib
from concourse.masks import make_identity
from concourse.tile_rust import add_dep_helper

P = 128
N_CORES = 8
TILES_PER_CALL = 8  # gather granularity; 1024 idxs = max per dma_gather call
SPLIT_ROWS_DEFAULT = 32640  # table-A real rows (255 chunks); A size 32768


def _wrap_idx16(linear_idx):
    """[n] int -> [128, n/16] int16 in the 16-partition wrapped, 8x
    replicated layout dma_gather expects (slot i at [i%16, i//16])."""
    n = linear_idx.shape[0]
    assert n % 16 == 0
    w = linear_idx.reshape(-1, 16).T.astype(np.int16)  # [16, n/16]
    return np.tile(w, (8, 1))


# ----------------------------------------------------------------------------
# Host-side layout construction (sharding / index relabeling only - all
# floating-point math happens on device).
# ----------------------------------------------------------------------------
def _prep(x, edge_index, weight, bias, n_cores, split_rows=SPLIT_ROWS_DEFAULT):
    N, D = x.shape
    assert D == P
    src = np.asarray(edge_index[0], dtype=np.int64)
    dst = np.asarray(edge_index[1], dtype=np.int64)
    E = src.shape[0]

    deg = np.bincount(dst, minlength=N)
    count = deg + 1  # self-loop included in the normalization counts

    CH = N // P + 1  # staged chunks; >=1 trailing zero row
    NSTAGED = CH * P

    two_tables = split_rows < N
    if two_tables:
        SPLIT = split_rows
        assert SPLIT % P == 0 and SPLIT + P <= 32768
        prelim = np.argsort(count, kind="stable")
        in_A = np.zeros(N, bool)
        in_A[prelim[:SPLIT]] = True
        # per-node count of A-source edges (self-loops handled separately)
        cntA = np.bincount(dst[in_A[src]], minlength=N)
        cntB_pre = deg - cntA
        snake = np.where(cntA % 2 == 0, cntB_pre, (1 << 20) - cntB_pre)
        order = np.lexsort((snake, cntA, ~in_A))
        NA = SPLIT + P  # pad rows SPLIT..NA-1 are zero
        NB = NSTAGED - SPLIT
    else:
        SPLIT = NSTAGED  # everything in table A
        in_A = np.ones(N, bool)
        cntA = deg.copy()
        order = np.argsort(count, kind="stable")
        NA = NSTAGED
        NB = 0
    cntB = deg - cntA

    rank = np.empty(N, np.int64)
    rank[order] = np.arange(N)

    count_staged = np.zeros(NSTAGED, np.int64)
    count_staged[:N] = count[order]
    cntA_staged = np.zeros(NSTAGED, np.int64)
    cntA_staged[:N] = cntA[order]
    cntB_staged = np.zeros(NSTAGED, np.int64)
    cntB_staged[:N] = cntB[order]
    x_staged = np.zeros((NSTAGED, D), np.float32)
    x_staged[:N] = np.asarray(x, dtype=np.float32)[order]

    # edges grouped by dst rank, A-sources first within each dst
    drank = rank[dst]
    src_in_B = ~in_A[src]
    eorder = np.lexsort((src_in_B, drank))
    esrc_rank = rank[src[eorder]]  # staged row of each message source
    deg_by_rank = (count[order] - 1).astype(np.int64)
    starts = np.zeros(N + 1, np.int64)
    starts[1:] = np.cumsum(deg_by_rank)

    LOCAL = (N + n_cores - 1) // n_cores
    GROUPS = (LOCAL + P - 1) // P
    LOCAL_PAD = GROUPS * P

    TgA, TgB = [], []
    for g in range(GROUPS):
        lo = n_cores * P * g
        hi = min(n_cores * P * (g + 1), N)
        if lo < N:
            TgA.append(int(cntA_staged[lo:hi].max()))
            TgB.append(int(cntB_staged[lo:hi].max()))
        else:
            TgA.append(1)
            TgB.append(0)
        if TgA[-1] + TgB[-1] == 0:
            TgA[-1] = 1
    toffsA = np.zeros(GROUPS + 1, np.int64)
    toffsA[1:] = np.cumsum(TgA)
    toffsB = np.zeros(GROUPS + 1, np.int64)
    toffsB[1:] = np.cumsum(TgB)
    T_totalA = int(toffsA[-1])
    T_totalB = int(toffsB[-1])

    # pad slots point at guaranteed-zero rows
    PAD_A = SPLIT if two_tables else N
    PAD_B = N - SPLIT if two_tables else 0  # staged zero tail (B rows)

    x_own = np.zeros((n_cores, GROUPS * P, D), np.float32)
    for c in range(n_cores):
        k = np.arange(min((N - c + n_cores - 1) // n_cores, GROUPS * P))
        s_r = n_cores * k + c
        x_own[c][: k.shape[0]] = x_staged[s_r]
    # [core, slot, group*feat] layout for the single big own-rows DMA
    x_own = np.ascontiguousarray(
        x_own.reshape(n_cores, GROUPS, P, D).transpose(0, 2, 1, 3)
    ).reshape(n_cores, P, GROUPS * D)

    idxA_cores = np.empty((n_cores, P, 8 * max(T_totalA, 1)), np.int16)
    idxB_cores = np.empty((n_cores, P, 8 * max(T_totalB, 1)), np.int16)
    cntsA_cores = np.ones((n_cores, P, max(T_totalA, 1)), np.float32)
    cntsB_cores = np.ones((n_cores, P, max(T_totalB, 1)), np.float32)
    cntl_cores = np.ones((n_cores, P, GROUPS), np.float32)
    prange = np.arange(P)

    for c in range(n_cores):
        linA = np.full(T_totalA * P, PAD_A, np.int64)
        linB = np.full(T_totalB * P, PAD_B, np.int64)
        for g in range(GROUPS):
            s = n_cores * (P * g + prange) + c  # global ranks of this group
            valid = s < N
            sc = np.minimum(s, N - 1)
            ca = np.where(valid, cntA_staged[sc], 0)  # A-source edges
            cb = np.where(valid, cntB_staged[sc], 0)
            st = starts[sc]
            cntl_cores[c][:, g] = np.where(valid, count_staged[sc], 1)

            # ---- pass A block: tiles toffsA[g] .. +TgA[g]
            TA = TgA[g]
            if TA > 0:
                colsA = np.arange(TA)[None, :]
                pickA = st[:, None] + colsA
                takeA = (colsA < ca[:, None]) & valid[:, None]
                srcA = esrc_rank[np.minimum(pickA, max(E - 1, 0))]
                valsA = np.where(takeA, srcA, PAD_A)
                base = int(toffsA[g]) * P
                linA[base : base + TA * P] = valsA.T.ravel()  # tile-major
                cntsA_cores[c][:, int(toffsA[g]) : int(toffsA[g]) + TA] = np.where(
                    takeA, count_staged[np.minimum(srcA, NSTAGED - 1)], 1
                )

            # ---- pass B block
            TB = TgB[g]
            if TB > 0:
                colsB = np.arange(TB)[None, :]
                pickB = st[:, None] + ca[:, None] + colsB
                takeB = (colsB < cb[:, None]) & valid[:, None]
                srcB = esrc_rank[np.minimum(pickB, max(E - 1, 0))]
                valsB = np.where(takeB, srcB - SPLIT, PAD_B)
                base = int(toffsB[g]) * P
                linB[base : base + TB * P] = valsB.T.ravel()
                cntsB_cores[c][:, int(toffsB[g]) : int(toffsB[g]) + TB] = np.where(
                    takeB, count_staged[np.minimum(srcB, NSTAGED - 1)], 1
                )

        assert linA.min() >= 0 and linA.max() < NA
        idxA_cores[c] = _wrap_idx16(linA) if T_totalA else 0
        if T_totalB:
            assert linB.min() >= 0 and linB.max() < NB
            idxB_cores[c] = _wrap_idx16(linB)

    # f32 gather tables (A gets zeroed pad rows; B's tail is staged zeros)
    xA = np.zeros((NA, D), np.float32)
    xA[: min(SPLIT, NSTAGED)] = x_staged[: min(SPLIT, NSTAGED)]
    if two_tables:
        xB = x_staged[SPLIT:]
        assert xB.shape[0] == NB
    else:
        xB = np.zeros((P, D), np.float32)

    wT = np.ascontiguousarray(np.asarray(weight, dtype=np.float32).T)
    bias_col = np.asarray(bias, dtype=np.float32).reshape(P, 1)

    return dict(
        N=N,
        D=D,
        E=E,
        n_cores=n_cores,
        NSTAGED=NSTAGED,
        SPLIT=SPLIT,
        NA=NA,
        NB=NB,
        two_tables=two_tables,
        GROUPS=GROUPS,
        LOCAL=LOCAL,
        LOCAL_PAD=LOCAL_PAD,
        TgA=TgA,
        TgB=TgB,
        toffsA=toffsA,
        toffsB=toffsB,
        T_totalA=T_totalA,
        T_totalB=T_totalB,
        xA=xA,
        xB=xB,
        x_own=x_own,
        idxA_cores=idxA_cores,
        idxB_cores=idxB_cores,
        cntsA_cores=cntsA_cores,
        cntsB_cores=cntsB_cores,
        cntl_cores=cntl_cores,
        wT=wT,
        bias_col=bias_col,
        order=order,
    )


# ----------------------------------------------------------------------------
# Device program
# ----------------------------------------------------------------------------
def _build(L):
    NA, NB = L["NA"], L["NB"]
    GROUPS = L["GROUPS"]
    TgA, TgB = L["TgA"], L["TgB"]
    toffsA, toffsB = L["toffsA"], L["toffsB"]
    T_totalA, T_totalB = L["T_totalA"], L["T_totalB"]
    LOCAL_PAD = L["LOCAL_PAD"]
    f32 = mybir.dt.float32
    bf16 = mybir.dt.bfloat16
    i16 = mybir.dt.int16
    AF = mybir.ActivationFunctionType

    # Batch processing order: largest-work-first so the big PE chains drain
    # while gathers still stream. The LAST batch's gather calls are
    # prefetched at stream start (pinned pool) so the tail never waits on
    # the final ring drain.
    n_batches = (GROUPS + 3) // 4
    batch_work = [
        sum(TgA[g] + TgB[g] for g in range(4 * b, min(4 * b + 4, GROUPS)))
        for b in range(n_batches)
    ]
    border = sorted(range(n_batches), key=lambda b: -batch_work[b])
    # gather calls consumed by the two smallest (last-processed) batches:
    # issued mid-stream into a pinned pool so the tail never waits on the
    # final ring drain
    pin_keys = set()
    for fb in border[-2:]:
        for g in range(4 * fb, min(4 * fb + 4, GROUPS)):
            for pass_key, Tp, toffs in (("A", TgA[g], toffsA), ("B", TgB[g], toffsB)):
                for jj in range(Tp):
                    pin_keys.add((pass_key, (int(toffs[g]) + jj) // TILES_PER_CALL))

    nc = bacc.Bacc("TRN2", debug=False, num_devices=L["n_cores"], num_swdge_queues=4)
    xA_dram = nc.dram_tensor("xA", [NA, P], bf16, kind="ExternalInput")
    xB_dram = nc.dram_tensor("xB", [max(NB, P), P], bf16, kind="ExternalInput")
    idxA_dram = nc.dram_tensor(
        "idxA", [P, 8 * max(T_totalA, 1)], i16, kind="ExternalInput"
    )
    idxB_dram = nc.dram_tensor(
        "idxB", [P, 8 * max(T_totalB, 1)], i16, kind="ExternalInput"
    )
    cntsA_dram = nc.dram_tensor(
        "cntsA", [P, max(T_totalA, 1)], f32, kind="ExternalInput"
    )
    cntsB_dram = nc.dram_tensor(
        "cntsB", [P, max(T_totalB, 1)], f32, kind="ExternalInput"
    )
    cntl_dram = nc.dram_tensor("cntl", [P, GROUPS], f32, kind="ExternalInput")
    # host-pretransposed: [slot partition, group, feat] so one DMA with
    # 128 large contiguous descriptors stages every group's own-node rows
    xown_dram = nc.dram_tensor("x_own", [P, GROUPS * P], bf16, kind="ExternalInput")
    wT_dram = nc.dram_tensor("wT", [P, P], f32, kind="ExternalInput")
    bias_dram = nc.dram_tensor("bias_col", [P, 1], f32, kind="ExternalInput")
    out_dram = nc.dram_tensor("out", [P, LOCAL_PAD], f32, kind="ExternalOutput")

    with tile.TileContext(nc) as tc:
        with (
            tc.tile_pool(name="const", bufs=1) as cpool,
            tc.tile_pool(name="msgf", bufs=12) as mfpool,
            tc.tile_pool(name="msgb", bufs=16) as mbpool,
            tc.tile_pool(name="msgpin", bufs=max(len(pin_keys), 1)) as pinpool,
            tc.tile_pool(name="diag", bufs=6) as gpool,
            tc.tile_pool(name="uself", bufs=6) as uspool,
            tc.tile_pool(name="agg", bufs=4) as apool,
            tc.tile_pool(name="outs", bufs=2) as opool,
            tc.tile_pool(name="ps", bufs=5, space="PSUM") as pspool,
            tc.tile_pool(name="ps2", bufs=2, space="PSUM") as ps2pool,
        ):
            lib_inst = nc.gpsimd.load_library(_mlp_lib)

            # ---- constant loads (gather-critical tables first)
            idxA_sb = cpool.tile([P, 8 * max(T_totalA, 1)], i16)
            nc.sync.dma_start(out=idxA_sb[:], in_=idxA_dram[:])
            idxB_sb = cpool.tile([P, 8 * max(T_totalB, 1)], i16)
            nc.sync.dma_start(out=idxB_sb[:], in_=idxB_dram[:])
            # small constant loads go on the scalar queue so they don't
            # inflate the sync-queue DMA semaphore the first gather waits on
            cntsA_sb = cpool.tile([P, max(T_totalA, 1)], f32)
            nc.scalar.dma_start(out=cntsA_sb[:], in_=cntsA_dram[:])
            cntsB_sb = cpool.tile([P, max(T_totalB, 1)], f32)
            nc.scalar.dma_start(out=cntsB_sb[:], in_=cntsB_dram[:])
            cntl_sb = cpool.tile([P, GROUPS], f32)
            nc.scalar.dma_start(out=cntl_sb[:], in_=cntl_dram[:])
            wT_sb = cpool.tile([P, P], f32)
            nc.scalar.dma_start(out=wT_sb[:], in_=wT_dram[:])
            bias_sb = cpool.tile([P, 1], f32)
            nc.scalar.dma_start(out=bias_sb[:], in_=bias_dram[:])
            ident_sb = cpool.tile([P, P], f32)
            make_identity(nc, ident_sb[:])

            # ---- rsqrt of integer-valued count tables (all float math here)
            normA_sb = cpool.tile([P, max(T_totalA, 1)], f32)
            nc.scalar.sqrt(normA_sb[:], cntsA_sb[:])
            nc.vector.reciprocal(normA_sb[:], normA_sb[:])
            normB_sb = cpool.tile([P, max(T_totalB, 1)], f32)
            nc.scalar.sqrt(normB_sb[:], cntsB_sb[:])
            nc.vector.reciprocal(normB_sb[:], normB_sb[:])
            dinvl_sb = cpool.tile([P, GROUPS], f32)
            nc.scalar.sqrt(dinvl_sb[:], cntl_sb[:])
            nc.vector.reciprocal(dinvl_sb[:], dinvl_sb[:])

            # own-node rows: emitted after the idx loads so its 1.6MB burst
            # doesn't delay the first gather's idx dependency
            xoall_sb = cpool.tile([P, GROUPS, P], bf16)
            nc.sync.dma_start(
                out=xoall_sb[:],
                in_=xown_dram[:, :].rearrange("p (g f) -> p g f", f=P),
            )

            # ---- gather + fused scale/convert + segment-sum (PE) + linear
            msg_tiles = {}
            qrr = [0]

            def ensure_call(pass_key, k):
                key = (pass_key, k)
                if key in msg_tiles:
                    return
                T_tot = T_totalA if pass_key == "A" else T_totalB
                x_src = xA_dram if pass_key == "A" else xB_dram
                idx_sb = idxA_sb if pass_key == "A" else idxB_sb
                norm_sb = normA_sb if pass_key == "A" else normB_sb
                t0 = k * TILES_PER_CALL
                cnt = min(TILES_PER_CALL, T_tot - t0)
                mf = mfpool.tile([P, TILES_PER_CALL, P], bf16)
                g_inst = nc.gpsimd.dma_gather(
                    mf[:, :cnt, :],
                    x_src[:, :],
                    idx_sb[:, 8 * t0 : 8 * (t0 + cnt)],
                    cnt * P,
                    cnt * P,
                    P,
                    queue_num=qrr[0] % 4,
                )
                qrr[0] += 1
                add_dep_helper(g_inst.ins, lib_inst.ins, reason="ucode lib before gather")
                pool = pinpool if key in pin_keys else mbpool
                mb = pool.tile([P, TILES_PER_CALL, P], bf16)
                nc.vector.tensor_tensor(
                    out=mb[:, :cnt, :],
                    in0=mf[:, :cnt, :],
                    in1=norm_sb[:, t0 : t0 + cnt].broadcast_to([P, cnt, P]),
                    op=mybir.AluOpType.mult,
                )
                msg_tiles[key] = mb

            pin_issue_at = max(1, n_batches // 2)
            for bi, b in enumerate(border):
                if bi == pin_issue_at:
                    for key in sorted(pin_keys):
                        ensure_call(*key)
                glo = 4 * b
                ghi = min(4 * b + 4, GROUPS)
                out_t = opool.tile([P, 4 * P], f32)
                for g in range(glo, ghi):
                    diag = gpool.tile([P, P], bf16)
                    nc.scalar.mul(diag[:], ident_sb[:], dinvl_sb[:, g : g + 1])
                    uself = uspool.tile([P, P], bf16)
                    nc.scalar.mul(
                        uself[:], xoall_sb[:, g, :], dinvl_sb[:, g : g + 1]
                    )
                    psum = pspool.tile([P, P], f32)
                    j = 0
                    for pass_key, Tp, toffs in (
                        ("A", TgA[g], toffsA),
                        ("B", TgB[g], toffsB),
                    ):
                        for jj in range(Tp):
                            t = int(toffs[g]) + jj
                            k, kk = divmod(t, TILES_PER_CALL)
                            ensure_call(pass_key, k)
                            nc.tensor.matmul(
                                out=psum[:],
                                lhsT=msg_tiles[(pass_key, k)][:, kk, :],
                                rhs=diag[:],
                                start=(j == 0),
                                stop=False,
                            )
                            j += 1
                    nc.tensor.matmul(
                        out=psum[:],
                        lhsT=uself[:],
                        rhs=diag[:],
                        start=(j == 0),
                        stop=True,
                    )
                    agg = apool.tile([P, P], f32)
                    nc.scalar.copy(out=agg[:], in_=psum[:])
                    psum2 = ps2pool.tile([P, P], f32)
                    nc.tensor.matmul(
                        out=psum2[:], lhsT=wT_sb[:], rhs=agg[:], start=True, stop=True
                    )
                    nc.scalar.activation(
                        out_t[:, (g - glo) * P : (g - glo + 1) * P],
                        psum2[:],
                        AF.Identity,
                        bias=bias_sb[:, 0:1],
                    )
                w = (ghi - glo) * P
                nc.sync.dma_start(
                    out=out_dram[:, glo * P : glo * P + w],
                    in_=out_t[:, :w],
                )

    nc.compile()
    return nc


def _in_maps(L):
    xA_b = L["xA"].astype(_BF16)
    xB_b = L["xB"].astype(_BF16)
    maps = []
    for c in range(L["n_cores"]):
        maps.append(
            {
                "xA": xA_b,
                "xB": xB_b,
                "x_own": L["x_own"][c].astype(_BF16),
                "idxA": L["idxA_cores"][c],
                "idxB": L["idxB_cores"][c],
                "cntsA": L["cntsA_cores"][c],
                "cntsB": L["cntsB_cores"][c],
                "cntl": L["cntl_cores"][c],
                "wT": L["wT"],
                "bias_col": L["bias_col"],
            }
        )
    return maps


def _assemble(L, outs):
    N = L["N"]
    n_cores = L["n_cores"]
    LOCAL = L["LOCAL"]
    order = L["order"]
    res = np.empty((N, P), np.float32)
    for c in range(n_cores):
        oc = np.asarray(outs[c]["out"])  # [128, LOCAL_PAD]
        k = np.arange(LOCAL)
        s = n_cores * k + c
        m = s < N
        res[order[s[m]]] = oc[:, :LOCAL][:, m].T
    return res


_CACHE = {}
LAST_EXEC_NS = None


def kernel(x, edge_index, weight, bias, *, trace=False, n_cores=N_CORES):
    global LAST_EXEC_NS
    x = np.asarray(x, dtype=np.float32)
    edge_index = np.asarray(edge_index)
    weight = np.asarray(weight, dtype=np.float32)
    bias = np.asarray(bias, dtype=np.float32)

    key = hash(edge_index.tobytes()) ^ hash((x.shape, n_cores))
    if key in _CACHE:
        L, nc = _CACHE[key]
        xs = np.zeros((L["NSTAGED"], P), np.float32)
        xs[: L["N"]] = x[L["order"]]
        xA = np.zeros((L["NA"], P), np.float32)
        xA[: L["SPLIT"]] = xs[: L["SPLIT"]]
        L["xA"] = xA
        L["xB"] = xs[L["SPLIT"] :] if L["two_tables"] else L["xB"]
        GP = L["GROUPS"] * P
        n_cores = L["n_cores"]
        x_own = np.zeros((n_cores, GP, P), np.float32)
        for c in range(n_cores):
            k = np.arange(min((L["N"] - c + n_cores - 1) // n_cores, GP))
            x_own[c][: k.shape[0]] = xs[n_cores * k + c]
        L["x_own"] = np.ascontiguousarray(
            x_own.reshape(n_cores, L["GROUPS"], P, P).transpose(0, 2, 1, 3)
        ).reshape(n_cores, P, GP)
        L["wT"] = np.ascontiguousarray(weight.T)
        L["bias_col"] = bias.reshape(P, 1)
    else:
        L = _prep(x, edge_index, weight, bias, n_cores)
        nc = _build(L)
        _CACHE.clear()
        _CACHE[key] = (L, nc)

    res = run_bass_kernel_spmd(
        nc, _in_maps(L), core_ids=list(range(n_cores)), trace=trace
    )
    LAST_EXEC_NS = res.exec_time_ns
    return _assemble(L, res.results)


# revision 63
# speedup vs baseline: 1.0425x; 1.0425x over previous
"""GCNConv (message passing + linear) on 8 Trainium2 NeuronCores.

Strategy (graph/data parallel, per sharding hint):
  - Nodes sorted by (table, A-count, snake) and dealt round-robin to the
    8 cores (core c owns sorted-ranks {s : s % 8 == c}).
  - Node features staged as two bf16 DRAM tables (int16 gather index
    limit); each core bulk row-gathers per-edge source rows with the Q7
    dma_gather instruction (256B descriptors, 1024 idx/call, 4 SWDGE
    queues), then fuses the rsqrt(deg_src) scale with the cast to the
    message tiles on the DVE (per-edge degree counts uploaded as
    integer-valued tables; all rsqrt/normalization math on device).
  - Segment-sum on the TensorEngine: message tile [128 slots, 128 feat]
    (stationary) x diagonal rsqrt(deg_dst) matrix (streaming)
    accumulated into PSUM - one slot per owned dst per tile; self-loop
    contributions enter via a sequentially-loaded x_own tile (keeps
    them off the gather path).
  - Group batches processed largest-count-first so the long PE chains
    drain while gathers still stream.
  - Final linear via W^T matmul + bias; output is [d_out, local_dst];
    host unpermutes/transposes back to [N, d_out].

The Bass program is rebuilt per distinct edge_index (layout constants are
baked into the instruction stream); all 8 cores share one program and
differ only in their input data.
"""

import ml_dtypes
import numpy as np

_BF16 = ml_dtypes.bfloat16

import concourse.bacc as bacc
import concourse.mybir as mybir
import concourse.tile as tile
from concourse.bass_utils import run_bass_kernel_spmd
from concourse.library_config import mlp as _mlp_lib
from concourse.masks import make_identity
from concourse.tile_rust import add_dep_helper

P = 128
N_CORES = 8
TILES_PER_CALL = 8  # gather granularity; 1024 idxs = max per dma_gather call
SPLIT_ROWS_DEFAULT = 32640  # table-A real rows (255 chunks); A size 32768


def _wrap_idx16(linear_idx):
    """[n] int -> [128, n/16] int16 in the 16-partition wrapped, 8x
    replicated layout dma_gather expects (slot i at [i%16, i//16])."""
    n = linear_idx.shape[0]
    assert n % 16 == 0
    w = linear_idx.reshape(-1, 16).T.astype(np.int16)  # [16, n/16]
    return np.tile(w, (8, 1))


# ----------------------------------------------------------------------------
# Host-side layout construction (sharding / index relabeling only - all
# floating-point math happens on device).
# ----------------------------------------------------------------------------
def _prep(x, edge_index, weight, bias, n_cores, split_rows=SPLIT_ROWS_DEFAULT):
    N, D = x.shape
    assert D == P
    src = np.asarray(edge_index[0], dtype=np.int64)
    dst = np.asarray(edge_index[1], dtype=np.int64)
    E = src.shape[0]

    deg = np.bincount(dst, minlength=N)
    count = deg + 1  # self-loop included in the normalization counts

    CH = N // P + 1  # staged chunks; >=1 trailing zero row
    NSTAGED = CH * P

    two_tables = split_rows < N
    if two_tables:
        SPLIT = split_rows
        assert SPLIT % P == 0 and SPLIT + P <= 32768
        prelim = np.argsort(count, kind="stable")
        in_A = np.zeros(N, bool)
        in_A[prelim[:SPLIT]] = True
        # per-node count of A-source edges (self-loops handled separately)
        cntA = np.bincount(dst[in_A[src]], minlength=N)
        cntB_pre = deg - cntA
        snake = np.where(cntA % 2 == 0, cntB_pre, (1 << 20) - cntB_pre)
        order = np.lexsort((snake, cntA, ~in_A))
        NA = SPLIT + P  # pad rows SPLIT..NA-1 are zero
        NB = NSTAGED - SPLIT
    else:
        SPLIT = NSTAGED  # everything in table A
        in_A = np.ones(N, bool)
        cntA = deg.copy()
        order = np.argsort(count, kind="stable")
        NA = NSTAGED
        NB = 0
    cntB = deg - cntA

    rank = np.empty(N, np.int64)
    rank[order] = np.arange(N)

    count_staged = np.zeros(NSTAGED, np.int64)
    count_staged[:N] = count[order]
    cntA_staged = np.zeros(NSTAGED, np.int64)
    cntA_staged[:N] = cntA[order]
    cntB_staged = np.zeros(NSTAGED, np.int64)
    cntB_staged[:N] = cntB[order]
    x_staged = np.zeros((NSTAGED, D), np.float32)
    x_staged[:N] = np.asarray(x, dtype=np.float32)[order]

    # edges grouped by dst rank, A-sources first within each dst
    drank = rank[dst]
    src_in_B = ~in_A[src]
    eorder = np.lexsort((src_in_B, drank))
    esrc_rank = rank[src[eorder]]  # staged row of each message source
    deg_by_rank = (count[order] - 1).astype(np.int64)
    starts = np.zeros(N + 1, np.int64)
    starts[1:] = np.cumsum(deg_by_rank)

    LOCAL = (N + n_cores - 1) // n_cores
    GROUPS = (LOCAL + P - 1) // P
    LOCAL_PAD = GROUPS * P

    TgA, TgB = [], []
    for g in range(GROUPS):
        lo = n_cores * P * g
        hi = min(n_cores * P * (g + 1), N)
        if lo < N:
            TgA.append(int(cntA_staged[lo:hi].max()))
            TgB.append(int(cntB_staged[lo:hi].max()))
        else:
            TgA.append(1)
            TgB.append(0)
        if TgA[-1] + TgB[-1] == 0:
            TgA[-1] = 1
    toffsA = np.zeros(GROUPS + 1, np.int64)
    toffsA[1:] = np.cumsum(TgA)
    toffsB = np.zeros(GROUPS + 1, np.int64)
    toffsB[1:] = np.cumsum(TgB)
    T_totalA = int(toffsA[-1])
    T_totalB = int(toffsB[-1])

    # pad slots point at guaranteed-zero rows
    PAD_A = SPLIT if two_tables else N
    PAD_B = N - SPLIT if two_tables else 0  # staged zero tail (B rows)

    x_own = np.zeros((n_cores, GROUPS * P, D), np.float32)
    for c in range(n_cores):
        k = np.arange(min((N - c + n_cores - 1) // n_cores, GROUPS * P))
        s_r = n_cores * k + c
        x_own[c][: k.shape[0]] = x_staged[s_r]
    # [core, slot, group*feat] layout for the single big own-rows DMA
    x_own = np.ascontiguousarray(
        x_own.reshape(n_cores, GROUPS, P, D).transpose(0, 2, 1, 3)
    ).reshape(n_cores, P, GROUPS * D)

    idxA_cores = np.empty((n_cores, P, 8 * max(T_totalA, 1)), np.int16)
    idxB_cores = np.empty((n_cores, P, 8 * max(T_totalB, 1)), np.int16)
    cntsA_cores = np.ones((n_cores, P, max(T_totalA, 1)), np.float32)
    cntsB_cores = np.ones((n_cores, P, max(T_totalB, 1)), np.float32)
    cntl_cores = np.ones((n_cores, P, GROUPS), np.float32)
    prange = np.arange(P)

    for c in range(n_cores):
        linA = np.full(T_totalA * P, PAD_A, np.int64)
        linB = np.full(T_totalB * P, PAD_B, np.int64)
        for g in range(GROUPS):
            s = n_cores * (P * g + prange) + c  # global ranks of this group
            valid = s < N
            sc = np.minimum(s, N - 1)
            ca = np.where(valid, cntA_staged[sc], 0)  # A-source edges
            cb = np.where(valid, cntB_staged[sc], 0)
            st = starts[sc]
            cntl_cores[c][:, g] = np.where(valid, count_staged[sc], 1)

            # ---- pass A block: tiles toffsA[g] .. +TgA[g]
            TA = TgA[g]
            if TA > 0:
                colsA = np.arange(TA)[None, :]
                pickA = st[:, None] + colsA
                takeA = (colsA < ca[:, None]) & valid[:, None]
                srcA = esrc_rank[np.minimum(pickA, max(E - 1, 0))]
                valsA = np.where(takeA, srcA, PAD_A)
                base = int(toffsA[g]) * P
                linA[base : base + TA * P] = valsA.T.ravel()  # tile-major
                cntsA_cores[c][:, int(toffsA[g]) : int(toffsA[g]) + TA] = np.where(
                    takeA, count_staged[np.minimum(srcA, NSTAGED - 1)], 1
                )

            # ---- pass B block
            TB = TgB[g]
            if TB > 0:
                colsB = np.arange(TB)[None, :]
                pickB = st[:, None] + ca[:, None] + colsB
                takeB = (colsB < cb[:, None]) & valid[:, None]
                srcB = esrc_rank[np.minimum(pickB, max(E - 1, 0))]
                valsB = np.where(takeB, srcB - SPLIT, PAD_B)
                base = int(toffsB[g]) * P
                linB[base : base + TB * P] = valsB.T.ravel()
                cntsB_cores[c][:, int(toffsB[g]) : int(toffsB[g]) + TB] = np.where(
                    takeB, count_staged[np.minimum(srcB, NSTAGED - 1)], 1
                )

        assert linA.min() >= 0 and linA.max() < NA
        idxA_cores[c] = _wrap_idx16(linA) if T_totalA else 0
        if T_totalB:
            assert linB.min() >= 0 and linB.max() < NB
            idxB_cores[c] = _wrap_idx16(linB)

    # f32 gather tables (A gets zeroed pad rows; B's tail is staged zeros)
    xA = np.zeros((NA, D), np.float32)
    xA[: min(SPLIT, NSTAGED)] = x_staged[: min(SPLIT, NSTAGED)]
    if two_tables:
        xB = x_staged[SPLIT:]
        assert xB.shape[0] == NB
    else:
        xB = np.zeros((P, D), np.float32)

    wT = np.ascontiguousarray(np.asarray(weight, dtype=np.float32).T)
    bias_col = np.asarray(bias, dtype=np.float32).reshape(P, 1)

    return dict(
        N=N,
        D=D,
        E=E,
        n_cores=n_cores,
        NSTAGED=NSTAGED,
        SPLIT=SPLIT,
        NA=NA,
        NB=NB,
        two_tables=two_tables,
        GROUPS=GROUPS,
        LOCAL=LOCAL,
        LOCAL_PAD=LOCAL_PAD,
        TgA=TgA,
        TgB=TgB,
        toffsA=toffsA,
        toffsB=toffsB,
        T_totalA=T_totalA,
        T_totalB=T_totalB,
        xA=xA,
        xB=xB,
        x_own=x_own,
        idxA_cores=idxA_cores,
        idxB_cores=idxB_cores,
        cntsA_cores=cntsA_cores,
        cntsB_cores=cntsB_cores,
        cntl_cores=cntl_cores,
        wT=wT,
        bias_col=bias_col,
        order=order,
    )


# ----------------------------------------------------------------------------
# Device program
# ----------------------------------------------------------------------------
def _build(L):
    NA, NB = L["NA"], L["NB"]
    GROUPS = L["GROUPS"]
    TgA, TgB = L["TgA"], L["TgB"]
    toffsA, toffsB = L["toffsA"], L["toffsB"]
    T_totalA, T_totalB = L["T_totalA"], L["T_totalB"]
    LOCAL_PAD = L["LOCAL_PAD"]
    f32 = mybir.dt.float32
    bf16 = mybir.dt.bfloat16
    i16 = mybir.dt.int16
    AF = mybir.ActivationFunctionType

    # Batch processing order: largest-work-first so the big PE chains drain
    # while gathers still stream. The LAST batch's gather calls are
    # prefetched at stream start (pinned pool) so the tail never waits on
    # the final ring drain.
    n_batches = (GROUPS + 3) // 4
    batch_work = [
        sum(TgA[g] + TgB[g] for g in range(4 * b, min(4 * b + 4, GROUPS)))
        for b in range(n_batches)
    ]
    border = sorted(range(n_batches), key=lambda b: -batch_work[b])

    nc = bacc.Bacc("TRN2", debug=False, num_devices=L["n_cores"], num_swdge_queues=4)
    xA_dram = nc.dram_tensor("xA", [NA, P], bf16, kind="ExternalInput")
    xB_dram = nc.dram_tensor("xB", [max(NB, P), P], bf16, kind="ExternalInput")
    idxA_dram = nc.dram_tensor(
        "idxA", [P, 8 * max(T_totalA, 1)], i16, kind="ExternalInput"
    )
    idxB_dram = nc.dram_tensor(
        "idxB", [P, 8 * max(T_totalB, 1)], i16, kind="ExternalInput"
    )
    cntsA_dram = nc.dram_tensor(
        "cntsA", [P, max(T_totalA, 1)], f32, kind="ExternalInput"
    )
    cntsB_dram = nc.dram_tensor(
        "cntsB", [P, max(T_totalB, 1)], f32, kind="ExternalInput"
    )
    cntl_dram = nc.dram_tensor("cntl", [P, GROUPS], f32, kind="ExternalInput")
    # host-pretransposed: [slot partition, group, feat] so one DMA with
    # 128 large contiguous descriptors stages every group's own-node rows
    xown_dram = nc.dram_tensor("x_own", [P, GROUPS * P], bf16, kind="ExternalInput")
    wT_dram = nc.dram_tensor("wT", [P, P], f32, kind="ExternalInput")
    bias_dram = nc.dram_tensor("bias_col", [P, 1], f32, kind="ExternalInput")
    out_dram = nc.dram_tensor("out", [P, LOCAL_PAD], f32, kind="ExternalOutput")

    with tile.TileContext(nc) as tc:
        with (
            tc.tile_pool(name="const", bufs=1) as cpool,
            tc.tile_pool(name="msgf", bufs=12) as mfpool,
            tc.tile_pool(name="msgb", bufs=16) as mbpool,
            tc.tile_pool(name="diag", bufs=6) as gpool,
            tc.tile_pool(name="uself", bufs=6) as uspool,
            tc.tile_pool(name="agg", bufs=4) as apool,
            tc.tile_pool(name="outs", bufs=2) as opool,
            tc.tile_pool(name="ps", bufs=5, space="PSUM") as pspool,
            tc.tile_pool(name="ps2", bufs=2, space="PSUM") as ps2pool,
        ):
            lib_inst = nc.gpsimd.load_library(_mlp_lib)

            # ---- constant loads (gather-critical tables first)
            idxA_sb = cpool.tile([P, 8 * max(T_totalA, 1)], i16)
            nc.sync.dma_start(out=idxA_sb[:], in_=idxA_dram[:])
            idxB_sb = cpool.tile([P, 8 * max(T_totalB, 1)], i16)
            nc.sync.dma_start(out=idxB_sb[:], in_=idxB_dram[:])
            # small constant loads go on the scalar queue so they don't
            # inflate the sync-queue DMA semaphore the first gather waits on
            cntsA_sb = cpool.tile([P, max(T_totalA, 1)], f32)
            nc.scalar.dma_start(out=cntsA_sb[:], in_=cntsA_dram[:])
            cntsB_sb = cpool.tile([P, max(T_totalB, 1)], f32)
            nc.scalar.dma_start(out=cntsB_sb[:], in_=cntsB_dram[:])
            cntl_sb = cpool.tile([P, GROUPS], f32)
            nc.scalar.dma_start(out=cntl_sb[:], in_=cntl_dram[:])
            wT_sb = cpool.tile([P, P], f32)
            nc.scalar.dma_start(out=wT_sb[:], in_=wT_dram[:])
            bias_sb = cpool.tile([P, 1], f32)
            nc.scalar.dma_start(out=bias_sb[:], in_=bias_dram[:])
            ident_sb = cpool.tile([P, P], f32)
            make_identity(nc, ident_sb[:])

            # ---- rsqrt of integer-valued count tables (all float math here)
            normA_sb = cpool.tile([P, max(T_totalA, 1)], f32)
            nc.scalar.sqrt(normA_sb[:], cntsA_sb[:])
            nc.vector.reciprocal(normA_sb[:], normA_sb[:])
            normB_sb = cpool.tile([P, max(T_totalB, 1)], f32)
            nc.scalar.sqrt(normB_sb[:], cntsB_sb[:])
            nc.vector.reciprocal(normB_sb[:], normB_sb[:])
            dinvl_sb = cpool.tile([P, GROUPS], f32)
            nc.scalar.sqrt(dinvl_sb[:], cntl_sb[:])
            nc.vector.reciprocal(dinvl_sb[:], dinvl_sb[:])

            # own-node rows: emitted after the idx loads so its 1.6MB burst
            # doesn't delay the first gather's idx dependency
            xoall_sb = cpool.tile([P, GROUPS, P], bf16)
            nc.sync.dma_start(
                out=xoall_sb[:],
                in_=xown_dram[:, :].rearrange("p (g f) -> p g f", f=P),
            )

            # ---- gather + fused scale/convert + segment-sum (PE) + linear
            msg_tiles = {}
            qrr = [0]

            def ensure_call(pass_key, k):
                key = (pass_key, k)
                if key in msg_tiles:
                    return
                T_tot = T_totalA if pass_key == "A" else T_totalB
                x_src = xA_dram if pass_key == "A" else xB_dram
                idx_sb = idxA_sb if pass_key == "A" else idxB_sb
                norm_sb = normA_sb if pass_key == "A" else normB_sb
                t0 = k * TILES_PER_CALL
                cnt = min(TILES_PER_CALL, T_tot - t0)
                mf = mfpool.tile([P, TILES_PER_CALL, P], bf16)
                g_inst = nc.gpsimd.dma_gather(
                    mf[:, :cnt, :],
                    x_src[:, :],
                    idx_sb[:, 8 * t0 : 8 * (t0 + cnt)],
                    cnt * P,
                    cnt * P,
                    P,
                    queue_num=qrr[0] % 4,
                )
                qrr[0] += 1
                add_dep_helper(g_inst.ins, lib_inst.ins, reason="ucode lib before gather")
                mb = mbpool.tile([P, TILES_PER_CALL, P], bf16)
                nc.vector.tensor_tensor(
                    out=mb[:, :cnt, :],
                    in0=mf[:, :cnt, :],
                    in1=norm_sb[:, t0 : t0 + cnt].broadcast_to([P, cnt, P]),
                    op=mybir.AluOpType.mult,
                )
                msg_tiles[key] = mb

            for b in border:
                glo = 4 * b
                ghi = min(4 * b + 4, GROUPS)
                out_t = opool.tile([P, 4 * P], f32)
                for g in range(glo, ghi):
                    diag = gpool.tile([P, P], bf16)
                    nc.scalar.mul(diag[:], ident_sb[:], dinvl_sb[:, g : g + 1])
                    uself = uspool.tile([P, P], bf16)
                    nc.scalar.mul(
                        uself[:], xoall_sb[:, g, :], dinvl_sb[:, g : g + 1]
                    )
                    psum = pspool.tile([P, P], f32)
                    j = 0
                    for pass_key, Tp, toffs in (
                        ("A", TgA[g], toffsA),
                        ("B", TgB[g], toffsB),
                    ):
                        for jj in range(Tp):
                            t = int(toffs[g]) + jj
                            k, kk = divmod(t, TILES_PER_CALL)
                            ensure_call(pass_key, k)
                            nc.tensor.matmul(
                                out=psum[:],
                                lhsT=msg_tiles[(pass_key, k)][:, kk, :],
                                rhs=diag[:],
                                start=(j == 0),
                                stop=False,
                            )
                            j += 1
                    nc.tensor.matmul(
                        out=psum[:],
                        lhsT=uself[:],
                        rhs=diag[:],
                        start=(j == 0),
                        stop=True,
                    )
                    agg = apool.tile([P, P], f32)
                    nc.scalar.copy(out=agg[:], in_=psum[:])
                    psum2 = ps2pool.tile([P, P], f32)
                    nc.tensor.matmul(
                        out=psum2[:], lhsT=wT_sb[:], rhs=agg[:], start=True, stop=True
                    )
                    nc.scalar.activation(
                        out_t[:, (g - glo) * P : (g - glo + 1) * P],
                        psum2[:],
                        AF.Identity,
                        bias=bias_sb[:, 0:1],
                    )
                w = (ghi - glo) * P
                nc.sync.dma_start(
                    out=out_dram[:, glo * P : glo * P + w],
                    in_=out_t[:, :w],
                )

    nc.compile()
    return nc


def _in_maps(L):
    xA_b = L["xA"].astype(_BF16)
    xB_b = L["xB"].astype(_BF16)
    maps = []
    for c in range(L["n_cores"]):
        maps.append(
            {
                "xA": xA_b,
                "xB": xB_b,
                "x_own": L["x_own"][c].astype(_BF16),
                "idxA": L["idxA_cores"][c],
                "idxB": L["idxB_cores"][c],
                "cntsA": L["cntsA_cores"][c],
                "cntsB": L["cntsB_cores"][c],
                "cntl": L["cntl_cores"][c],
                "wT": L["wT"],
                "bias_col": L["bias_col"],
            }
        )
    return maps


def _assemble(L, outs):
    N = L["N"]
    n_cores = L["n_cores"]
    LOCAL = L["LOCAL"]
    order = L["order"]
    res = np.empty((N, P), np.float32)
    for c in range(n_cores):
        oc = np.asarray(outs[c]["out"])  # [128, LOCAL_PAD]
        k = np.arange(LOCAL)
        s = n_cores * k + c
        m = s < N
        res[order[s[m]]] = oc[:, :LOCAL][:, m].T
    return res


_CACHE = {}
LAST_EXEC_NS = None


def kernel(x, edge_index, weight, bias, *, trace=False, n_cores=N_CORES):
    global LAST_EXEC_NS
    x = np.asarray(x, dtype=np.float32)
    edge_index = np.asarray(edge_index)
    weight = np.asarray(weight, dtype=np.float32)
    bias = np.asarray(bias, dtype=np.float32)

    key = hash(edge_index.tobytes()) ^ hash((x.shape, n_cores))
    if key in _CACHE:
        L, nc = _CACHE[key]
        xs = np.zeros((L["NSTAGED"], P), np.float32)
        xs[: L["N"]] = x[L["order"]]
        xA = np.zeros((L["NA"], P), np.float32)
        xA[: L["SPLIT"]] = xs[: L["SPLIT"]]
        L["xA"] = xA
        L["xB"] = xs[L["SPLIT"] :] if L["two_tables"] else L["xB"]
        GP = L["GROUPS"] * P
        n_cores = L["n_cores"]
        x_own = np.zeros((n_cores, GP, P), np.float32)
        for c in range(n_cores):
            k = np.arange(min((L["N"] - c + n_cores - 1) // n_cores, GP))
            x_own[c][: k.shape[0]] = xs[n_cores * k + c]
        L["x_own"] = np.ascontiguousarray(
            x_own.reshape(n_cores, L["GROUPS"], P, P).transpose(0, 2, 1, 3)
        ).reshape(n_cores, P, GP)
        L["wT"] = np.ascontiguousarray(weight.T)
        L["bias_col"] = bias.reshape(P, 1)
    else:
        L = _prep(x, edge_index, weight, bias, n_cores)
        nc = _build(L)
        _CACHE.clear()
        _CACHE[key] = (L, nc)

    res = run_bass_kernel_spmd(
        nc, _in_maps(L), core_ids=list(range(n_cores)), trace=trace
    )
    LAST_EXEC_NS = res.exec_time_ns
    return _assemble(L, res.results)
